# revision 30
# baseline (speedup 1.0000x reference)
"""Trainium2 Bass/Tile kernel: 2-layer bidirectional LSTM encoder.

Contract: kernel(**inputs) takes the FULL unsharded inputs and returns the
full [T, B, 2H] output. Batch is split across 8 NeuronCores (data
parallel); weights replicated.

Shapes (hardcoded): T=160, B=256, C=512, H=256, G=4H=1024, 8 cores,
BC = 32 batch per core.

Per-core algorithm (v2):
 - x is preloaded to SBUF in full (bf16, feature-major, forward time
   order only), so layer 0's xg GEMM reads SBUF-resident operands.
 - xg is computed per-direction in "quads" = [128 rows = (4 steps,
   batch32), 1024 gates] fp32 PSUM tiles (M=128 GEMMs), bias folded in
   via a ones-row matmul, and STAYS in PSUM; the per-step recurrent
   matmuls accumulate into the same rows via has_written (start=False),
   so gates = xg + h@WhhT needs no DVE add, no gather DMA and no PSUM
   evacuation. Each region's first writer uses start=True (required:
   start=False on virgin rows over-accumulates on HW).
 - The two dirs' quads are staggered by 2 steps and share a 3-slot PSUM
   rotation whose creation order provably matches retirement order
   (warm order d1j0, d0j0, d1j1). Direction 1 reads x/h0 in true-time
   ascending pairs with a flipped row slot, so no reversed x copy.
 - The two directions run as independent software-pipelined chains
   (separate ACT/DVE calls, offset half a step) so ACT/DVE work of one
   dir overlaps PE work of the other.
 - Gate order is host-permuted to [i,f,o,g]: one sigmoid covers cols
   0:768 of a row block, one tanh 768:1024.
 - c stays fp32 (the accumulator); x, weights, h, activations, and the
   fc/ig products are bf16 (2x DVE modes).
 - h is transposed via PE transpose (bf16) into a small PSUM tile, then
   DVE-copied to a compact per-dir hT ping-pong (feeds the next step's
   recurrent matmul) and, on layer 0, also to the h0T history buffer
   (true-time layout, feeds layer 1's GEMM).
"""

import os
import sys

import numpy as np

for _p in ("/opt/trn_rl_repo", "/root/.axon_site/_ro/trn_rl_repo"):
    if os.path.isdir(_p) and _p not in sys.path:
        sys.path.insert(0, _p)

from contextlib import ExitStack

import concourse.bass as bass  # noqa: F401
import concourse.mybir as mybir
import concourse.tile as tile
from concourse import bacc, bass_utils

AF = mybir.ActivationFunctionType
F32 = mybir.dt.float32
F32R = mybir.dt.float32r
BF16 = mybir.dt.bfloat16

T, B, CIN, H = 160, 256, 512, 256
G = 4 * H  # 1024
HALF = G // 2  # 512
NCORES = 8
BC = B // NCORES  # 32
ND = T // 2  # 80 duos of 2 timesteps

# torch gate order [i,f,g,o] -> ours [i,f,o,g] (sigmoid block contiguous)
_PERM = np.concatenate(
    [np.arange(0, 512), np.arange(768, 1024), np.arange(512, 768)]
)

_CACHE = {}


def _build():
    nc = bacc.Bacc("TRN2", target_bir_lowering=False, debug=False)

    xT_d = nc.dram_tensor("xT", [CIN, T * BC], BF16, kind="ExternalInput").ap()
    wih_d = [
        [
            nc.dram_tensor(f"wih{l}{d}", [128, 4 * G], BF16, kind="ExternalInput").ap()
            for d in (0, 1)
        ]
        for l in (0, 1)
    ]
    whh_d = [
        [
            nc.dram_tensor(f"whh{l}{d}", [128, 2 * G], BF16, kind="ExternalInput").ap()
            for d in (0, 1)
        ]
        for l in (0, 1)
    ]
    bias_d = [
        [
            nc.dram_tensor(f"bias{l}{d}", [1, G], F32R, kind="ExternalInput").ap()
            for d in (0, 1)
        ]
        for l in (0, 1)
    ]
    ones_d = nc.dram_tensor("ones", [1, 128], F32R, kind="ExternalInput").ap()
    identT_d = nc.dram_tensor("identT", [32, 32], BF16, kind="ExternalInput").ap()
    out_d = nc.dram_tensor("out", [T, BC, 2 * H], BF16, kind="ExternalOutput").ap()

    with tile.TileContext(nc) as tc, ExitStack() as ctx:
        # PSUM: quad pool first (3 x 4KB -> banks 0-5), trp pool (banks 6-7)
        ps_q = ctx.enter_context(tc.tile_pool(name="ps_q", bufs=3, space="PSUM"))
        ps_t = ctx.enter_context(tc.tile_pool(name="ps_t", bufs=2, space="PSUM"))

        const = ctx.enter_context(tc.tile_pool(name="const", bufs=1))
        big = ctx.enter_context(tc.tile_pool(name="big", bufs=1))
        sb = ctx.enter_context(tc.tile_pool(name="sb", bufs=2))

        identT_sb = const.tile([32, 32], BF16)
        nc.sync.dma_start(identT_sb[:], identT_d[:])
        ones_sb = const.tile([1, 128], F32R)
        nc.sync.dma_start(ones_sb[:], ones_d[:])

        # full x resident in SBUF, feature-major (true time order):
        # 4 chunks of [128 cin, T*BC] bf16
        xT_sb = [big.tile([128, T * BC], BF16, name=f"xT{ki}s") for ki in range(4)]
        for ki in range(4):
            nc.sync.dma_start(xT_sb[ki][:], xT_d[ki * 128 : (ki + 1) * 128, :])

        # h0T: layer-0 output, feature-major, TRUE time for both dirs:
        # [128, (k=2, dir=2, t=T, b=32)]
        h0T = big.tile([128, 2 * T * 64], BF16)
        h0T_r = h0T[:].rearrange("p (k dd t b) -> p k dd t b", k=2, dd=2, t=T)

        layers = (0,) if os.environ.get("DEBUG_L0") else (0, 1)
        for l in layers:
            wih_sb = [
                sb.tile([128, 4 * G], BF16, tag=f"wih{l}{d}", bufs=1, name=f"wih{l}{d}s")
                for d in (0, 1)
            ]
            whh_sb = [
                sb.tile([128, 2 * G], BF16, tag=f"whh{l}{d}", bufs=1, name=f"whh{l}{d}s")
                for d in (0, 1)
            ]
            bias_sb = [
                sb.tile([1, G], F32R, tag=f"bias{l}{d}", bufs=1, name=f"bias{l}{d}s")
                for d in (0, 1)
            ]
            for d in (0, 1):
                nc.sync.dma_start(wih_sb[d][:], wih_d[l][d][:])
                nc.sync.dma_start(whh_sb[d][:], whh_d[l][d][:])
                nc.sync.dma_start(bias_sb[d][:], bias_d[l][d][:])

            quads = {}  # (d, j) -> psum tile
            hT_prev = [None, None]

            def quad_t0_m(d, j):
                # starting true-time column and row count of quad (d, j)
                if d == 0:
                    return 4 * j, 128
                if j == 0:
                    return T - 2, 64
                if j == 40:
                    return 0, 64
                return T - 2 - 4 * j, 128

            def gemm_lhsT(d, j, ki, m, l=l):
                t0, _ = quad_t0_m(d, j)
                if l == 0:
                    return xT_sb[ki][:, t0 * 32 : t0 * 32 + m]
                k, dsrc = ki % 2, ki // 2
                base = (k * 2 + dsrc) * T * 32 + t0 * 32
                return h0T[:, base : base + m]

            def emit_gemm_chunk(d, j, part, wih_sb=wih_sb, bias_sb=bias_sb,
                                quads=quads):
                # part 0: ki 0,1 + bias nh0; part 1: ki 2,3 + bias nh1.
                _, m = quad_t0_m(d, j)
                if part == 0:
                    quads[(d, j)] = ps_q.tile(
                        [128, G], F32, tag="quad", name=f"q{l}{d}_{j}"
                    )
                xg_ps = quads[(d, j)]
                for ki in (0, 1) if part == 0 else (2, 3):
                    lhsT = gemm_lhsT(d, j, ki, m)
                    for nh in (0, 1):
                        nc.tensor.matmul(
                            xg_ps[0:m, nh * HALF : (nh + 1) * HALF],
                            lhsT,
                            wih_sb[d][
                                :, ki * G + nh * HALF : ki * G + (nh + 1) * HALF
                            ],
                            start=(ki == 0),
                            stop=False,
                            skip_group_check=True,
                            tile_position=(0, 0),
                        )
                nh = part
                nc.tensor.matmul(
                    xg_ps[0:m, nh * HALF : (nh + 1) * HALF],
                    ones_sb[:, 0:m],
                    bias_sb[d][:, nh * HALF : (nh + 1) * HALF],
                    start=False,
                    stop=(j == 0),
                    skip_group_check=True,
                )

            # warm quads in retirement order (d1 j0 retires first at s=1,
            # then d0 j0 at s=3, then d1 j1 at s=5) so the 3-slot
            # round-robin always reuses the just-retired slot.
            for d0j in ((1, 0), (0, 0), (1, 1)):
                for part in (0, 1):
                    emit_gemm_chunk(d0j[0], d0j[1], part)

            c_prev = [None, None]
            hT_cur = [None, None]

            def rec(d, s):
                m4 = s % 4
                if d == 0:
                    j, slot = s // 4, m4
                else:
                    j = min((s + 2) // 4, 40)
                    slot = (4 * j + 1 - s) if j <= 39 else (159 - s)
                xg_ps = quads[(d, j)]
                rows = slice(32 * slot, 32 * slot + 32)
                with tc.high_priority(offset=60):
                    emit_rec_mms(d, s, xg_ps, rows, slot)

            def emit_rec_mms(d, s, xg_ps, rows, slot):
                for nh in (0, 1):
                    for k in (0, 1):
                        lhsT_h = hT_prev[d][:, k * 32 : k * 32 + 32]
                        nc.tensor.matmul(
                            xg_ps[rows, nh * HALF : (nh + 1) * HALF],
                            lhsT_h,
                            whh_sb[d][
                                :, k * G + nh * HALF : k * G + (nh + 1) * HALF
                            ],
                            start=False,
                            stop=(k == 1),
                            skip_group_check=True,
                            tile_position=(0, 32 * slot),
                        )

            def chain(d, s):
                # full per-dir elementwise chain: sigmoid/tanh_g -> c ->
                # tanh_c -> h -> transpose/evacuate (+ output DMA on l1)
                m4 = s % 4
                if d == 0:
                    j, slot = s // 4, m4
                else:
                    j = min((s + 2) // 4, 40)
                    slot = (4 * j + 1 - s) if j <= 39 else (159 - s)
                xg_ps = quads[(d, j)]
                rows = slice(32 * slot, 32 * slot + 32)
                t_true = s if d == 0 else T - 1 - s

                ifo = sb.tile([32, 768], BF16, tag=f"ifo{d}", bufs=4,
                              name=f"ifo{l}{d}_{s}")
                nc.scalar.activation(ifo[:], xg_ps[rows, 0:768], AF.Sigmoid)
                gt = sb.tile([32, 256], BF16, tag=f"gt{d}", bufs=4,
                             name=f"gt{l}{d}_{s}")
                nc.scalar.activation(gt[:], xg_ps[rows, 768:1024], AF.Tanh)

                if s == 0:
                    c_new = sb.tile([32, 256], F32, tag=f"c{d}", bufs=3,
                                    name=f"c{l}{d}_{s}")
                    nc.vector.tensor_mul(c_new[:], ifo[:, 0:256], gt[:])
                else:
                    fc = sb.tile([32, 256], BF16, tag=f"fc{d}",
                                 name=f"fc{l}{d}_{s}")
                    nc.vector.tensor_mul(fc[:], ifo[:, 256:512], c_prev[d][:])
                    ig = sb.tile([32, 256], BF16, tag=f"ig{d}",
                                 name=f"ig{l}{d}_{s}")
                    nc.vector.tensor_mul(ig[:], ifo[:, 0:256], gt[:])
                    c_new = sb.tile([32, 256], F32, tag=f"c{d}", bufs=3,
                                    name=f"cn{l}{d}_{s}")
                    nc.vector.tensor_add(c_new[:], fc[:], ig[:])
                c_prev[d] = c_new

                tct = sb.tile([32, 256], BF16, tag=f"tct{d}", bufs=3,
                              name=f"tct{l}{d}_{s}")
                nc.scalar.activation(tct[:], c_new[:], AF.Tanh)
                h = sb.tile([32, 256], BF16, tag=f"h{d}", bufs=4,
                            name=f"h{l}{d}_{s}")
                nc.vector.tensor_mul(h[:], ifo[:, 512:768], tct[:])

                if l == 0:
                    trp = ps_t.tile([128, 64], BF16, tag="trp",
                                    name=f"trp{l}{d}_{s}")
                    for k in (0, 1):
                        nc.tensor.transpose(
                            trp[:, k * 32 : (k + 1) * 32],
                            h[:, k * 128 : (k + 1) * 128],
                            identT_sb[:],
                        )
                    hT_new = sb.tile([128, 64], BF16, tag=f"hT{d}",
                                     bufs=3, name=f"hTa{l}{d}_{s}")
                    nc.vector.tensor_copy(hT_new[:], trp[:])
                    hT_cur[d] = hT_new
                    nc.vector.tensor_copy(
                        h0T_r[:, :, d, t_true, :],
                        trp[:].rearrange("p (k b) -> p k b", k=2),
                    )
                else:
                    nc.sync.dma_start(
                        out_d[t_true, :, 256 * d : 256 * d + 256], h[:]
                    )
                    if s < T - 1:
                        trp = ps_t.tile([128, 64], BF16, tag="trp",
                                        name=f"trp{l}{d}_{s}")
                        for k in (0, 1):
                            nc.tensor.transpose(
                                trp[:, k * 32 : (k + 1) * 32],
                                h[:, k * 128 : (k + 1) * 128],
                                identT_sb[:],
                            )
                        hT_new = sb.tile([128, 64], BF16, tag=f"hT{d}",
                                         bufs=3, name=f"hT{l}{d}_{s}")
                        nc.vector.tensor_copy(hT_new[:], trp[:])
                        hT_cur[d] = hT_new

            def gemm_sched(s):
                # one 5-MM chunk per step: d0 quad j+1 at steps 4j+2/4j+3,
                # d1 quad j+1 at steps 4j/4j+1 (j>=1).
                m4 = s % 4
                if m4 in (2, 3) and s // 4 + 1 <= 39:
                    emit_gemm_chunk(0, s // 4 + 1, m4 - 2)
                elif m4 in (0, 1) and s >= 4 and s // 4 + 1 <= 40:
                    emit_gemm_chunk(1, s // 4 + 1, m4)

            def gemm_sched_early(s):
                pass

            # software-pipelined scan: the two dir chains are offset by
            # half a step so ACT/DVE work of one dir overlaps PE work of
            # the other.
            for s in range(T):
                gemm_sched_early(s)
                if s > 0:
                    rec(1, s)
                gemm_sched(s)
                chain(0, s)
                hT_prev = list(hT_cur)
                if s + 1 < T:
                    rec(0, s + 1)
                chain(1, s)
                hT_prev = list(hT_cur)

    nc.compile()
    return nc


def _prep_inputs(inputs):
    import ml_dtypes

    bf = ml_dtypes.bfloat16
    x = np.asarray(inputs["x"], dtype=np.float32)
    common = {}
    for l in (0, 1):
        for d, sfx in enumerate(("", "_reverse")):
            Wih = np.asarray(inputs[f"weight_ih_l{l}{sfx}"], dtype=np.float32)
            Whh = np.asarray(inputs[f"weight_hh_l{l}{sfx}"], dtype=np.float32)
            bsum = (
                np.asarray(inputs[f"bias_ih_l{l}{sfx}"], dtype=np.float32)
                + np.asarray(inputs[f"bias_hh_l{l}{sfx}"], dtype=np.float32)
            )
            wihT = np.ascontiguousarray(Wih.T[:, _PERM])  # [512, 1024]
            whhT = np.ascontiguousarray(Whh.T[:, _PERM])  # [256, 1024]
            common[f"wih{l}{d}"] = (
                wihT.reshape(4, 128, G).transpose(1, 0, 2).reshape(128, 4 * G)
            ).astype(bf)
            common[f"whh{l}{d}"] = (
                whhT.reshape(2, 128, G).transpose(1, 0, 2).reshape(128, 2 * G)
            ).astype(bf)
            common[f"bias{l}{d}"] = bsum[_PERM][None, :]
    common["ones"] = np.ones((1, 128), dtype=np.float32)
    common["identT"] = np.eye(32, dtype=np.float32).astype(bf)

    in_maps = []
    for c in range(NCORES):
        xs = x[:, c * BC : (c + 1) * BC, :]  # [T, 32, 512]
        m = dict(common)
        m["xT"] = np.ascontiguousarray(
            xs.transpose(2, 0, 1).reshape(CIN, T * BC).astype(bf)
        )
        in_maps.append(m)
    return in_maps


def _get_program():
    if "prog" not in _CACHE:
        _CACHE["prog"] = _build()
    return _CACHE["prog"]


def kernel(**inputs):
    nc = _get_program()
    in_maps = _prep_inputs(inputs)
    res = bass_utils.run_bass_kernel_spmd(nc, in_maps, core_ids=list(range(NCORES)))
    out = np.empty((T, B, 2 * H), np.float32)
    for c in range(NCORES):
        out[:, c * BC : (c + 1) * BC, :] = res.results[c]["out"].astype(np.float32)
    return out
